# revision 15
# baseline (speedup 1.0000x reference)
"""Trainium2 Bass kernel for the LIF dense layer (spike output only).

The reference computes
    P_n   = quant8(alpha*P + Q)            (grid 1/128, round-half-even)
    U     = P_n @ quant8(W) + quant8(b) - S
    S_n   = (U > 0.4)
``input_t`` and ``R`` never influence the output (Q_n/U_q are dead,
gamma == 0), so they are never loaded.

All quantized operands are 8-bit integers scaled by 1/128, hence exactly
representable in bf16, and every partial matmul sum is a multiple of 2^-14
below 2^24 -> bf16 matmul with fp32 PSUM accumulation is bit-exact vs the
fp32 reference einsum.  Rounding uses the fp32 magic-number trick
(x + 1.5*2^16) - 1.5*2^16 == round-to-nearest-even onto the 1/128 grid.

Everything runs transposed: the small [512,512] W is the matmul's
stationary operand, P_n^T the moving one, so PSUM holds U^T tiles
[128 o, nb b] and the per-output constant C = 0.4 - b_q becomes a
per-PARTITION scalar.  The whole epilogue is then a single fused DVE op
per tile:  spike^T = (U^T - C_col) is_gt S^T  (subtract + compare +
uint8 narrowing in one pass).  U^T is exact on the 2^-14 grid, C is one
f32 rounding off the real value (< 1e-7), and 0.4f+k*2^-7 sits 2.4e-5
away from the nearest grid point, so decisions match the reference
bit-for-bit (b_q is bf16-exact: k/128, |k|<=127).

The quantizer's clip at +/-127/128 only matters when |alpha*P+Q| >=
127.5/128; the host checks the actual inputs (one cheap fused scan) and
compiles the clip pass in only when needed.

Host-side prep: P, Q, S are shipped transposed and chunk-blocked
([nchunks, 128, ., nb] per core) so the contraction (resp. output) dim
lands on SBUF partitions directly -- no on-chip transposes and 4-16 KiB
contiguous per partition per DMA.  S travels as uint8 and the spike
output returns as blocked uint8 (values are exactly 0/1), quartering
their HBM traffic.

Engine split per chunk: Pool computes alpha*P+Q (fused mult+add), ACT
does the two magic-number adds (the second narrowing to bf16), TensorE
runs 16 N=1024 matmuls, DVE runs the two fused spike ops off PSUM.

Sharding: pure data parallel over the batch dim, 4096 rows per core on 8
NeuronCores; the [512,512] weights / bias are quantized host-side (exact
replication of the reference quantizer) and replicated.
"""

import sys

import numpy as np

sys.path.insert(0, "/opt/trn_rl_repo")

import ml_dtypes

B, IN, OUT = 32768, 512, 512
NCORES = 8
BL = B // NCORES            # rows per core
PART = 128                  # SBUF partitions
KCH = IN // PART            # contraction chunks of 128
KOB = OUT // PART           # output chunks of 128
NB = 512                    # batch columns per pipeline chunk
NCHUNKS = BL // NB
OG = 2                      # o-blocks per PSUM group / fused spike op
# exp(-dt/tau_mem) as computed by XLA fp32 (1 ulp above numpy's expf)
ALPHA = float(np.array(1062312023, np.uint32).view(np.float32))
MAGIC = 98304.0             # 1.5*2^16: fp32 +/- rounds to multiples of 2^-7
QMAX = 127.0 / 128.0
THR = 0.4


def build_nc(bl=BL, nb=NB, clamp=False, enable_asserts=False):
    import concourse.bass as bass
    import concourse.bacc as bacc
    import concourse.mybir as mybir
    from concourse import tile

    OP = mybir.AluOpType
    AF = mybir.ActivationFunctionType
    dt = mybir.dt
    ts = bass.ts

    nchunks = bl // nb

    # Bacc (not plain Bass): its compile() splits multi-sem waits into
    # event semaphores -- TRN2 allows one wait per instruction.
    nc = bacc.Bacc(
        "TRN2",
        target_bir_lowering=False,
        debug=False,
        enable_asserts=enable_asserts,
        num_devices=NCORES,
    )
    p_d = nc.dram_tensor(
        "p", [nchunks, PART, KCH, nb], dt.float32, kind="ExternalInput"
    ).ap()
    q_d = nc.dram_tensor(
        "q", [nchunks, PART, KCH, nb], dt.float32, kind="ExternalInput"
    ).ap()
    s_d = nc.dram_tensor(
        "s", [nchunks, PART, KOB, nb], dt.uint8, kind="ExternalInput"
    ).ap()
    w_d = nc.dram_tensor(
        "w", [PART, KCH, KOB, PART], dt.bfloat16, kind="ExternalInput"
    ).ap()
    c_d = nc.dram_tensor("ccol", [PART, KOB], dt.float32,
                         kind="ExternalInput").ap()
    o_d = nc.dram_tensor(
        "o", [nchunks, PART, KOB, nb], dt.uint8, kind="ExternalOutput"
    ).ap()

    with tile.TileContext(nc) as tc:
        with (
            tc.tile_pool(name="const", bufs=1) as cpool,
            tc.tile_pool(name="io", bufs=4) as iop,
            tc.tile_pool(name="wk", bufs=3) as wkp,
            tc.tile_pool(name="ps", bufs=4, space="PSUM") as psp,
        ):
            w_sb = cpool.tile([PART, KCH, KOB, PART], dt.bfloat16)
            nc.sync.dma_start(out=w_sb[:], in_=w_d[:])
            ccol_sb = cpool.tile([PART, KOB], dt.float32)
            nc.sync.dma_start(out=ccol_sb[:], in_=c_d[:])
            magic_p = cpool.tile([PART, 1], dt.float32)
            nc.vector.memset(magic_p[:], MAGIC)
            magic_n = cpool.tile([PART, 1], dt.float32)
            nc.vector.memset(magic_n[:], -MAGIC)

            for c in range(nchunks):
                p_t = iop.tile([PART, KCH, nb], dt.float32, tag="p")
                q_t = iop.tile([PART, KCH, nb], dt.float32, tag="q")
                s_t = iop.tile([PART, KOB, nb], dt.uint8, tag="s")
                sp_t = iop.tile([PART, KOB, nb], dt.uint8, tag="sp")
                nc.sync.dma_start(out=p_t[:], in_=p_d[c])
                nc.scalar.dma_start(out=q_t[:], in_=q_d[c])
                nc.scalar.dma_start(out=s_t[:], in_=s_d[c])

                # x = alpha*P + Q  (fp32, per-stage rounding as in the ref)
                nc.vector.scalar_tensor_tensor(
                    out=p_t[:], in0=p_t[:], scalar=ALPHA, in1=q_t[:],
                    op0=OP.mult, op1=OP.add,
                )
                if clamp:
                    # the reference quantizer's clip at +/-127/128
                    nc.gpsimd.tensor_scalar(
                        out=p_t[:], in0=p_t[:], scalar1=QMAX, scalar2=-QMAX,
                        op0=OP.min, op1=OP.max,
                    )
                # round-half-even onto the 1/128 grid (magic-number trick);
                # k/128 is bf16-exact, so the second add narrows to bf16
                pn_t = wkp.tile([PART, KCH, nb], dt.bfloat16, tag="pn")
                nc.scalar.activation(
                    p_t[:], p_t[:], AF.Identity, bias=magic_p[:],
                )
                nc.scalar.activation(
                    pn_t[:], p_t[:], AF.Identity, bias=magic_n[:],
                )

                for g in range(KOB // OG):
                    u_ps = psp.tile([PART, OG, nb], dt.float32, tag="u")
                    for jj in range(OG):
                        j = g * OG + jj
                        for k in range(KCH):
                            # one PSUM bank (2 KiB) per matmul -> N=512 halves;
                            # consecutive halves share the stationary weights
                            for h in range(nb // 512):
                                nc.tensor.matmul(
                                    u_ps[:, jj, ts(h, 512)],
                                    lhsT=w_sb[:, k, j, :],
                                    rhs=pn_t[:, k, ts(h, 512)],
                                    start=(k == 0),
                                    stop=(k == KCH - 1),
                                )
                    # spike^T = (U^T - C_col) > S^T   (one fused DVE op)
                    for jj in range(OG):
                        j = g * OG + jj
                        nc.vector.scalar_tensor_tensor(
                            out=sp_t[:, j, :],
                            in0=u_ps[:, jj, :],
                            scalar=ccol_sb[:, j:j + 1],
                            in1=s_t[:, j, :],
                            op0=OP.subtract,
                            op1=OP.is_gt,
                        )
                nc.sync.dma_start(out=o_d[c], in_=sp_t[:])
    nc.finalize()  # Bacc.compile(): splits multi-sem waits (TRN2 1-wait rule)
    return nc


def _quant_host(x):
    """Exact replica of the reference quant_ste forward pass (fp32)."""
    x = np.asarray(x, np.float32)
    d = np.float32(1.0) / np.float32(128.0)
    y = np.clip(x, np.float32(-1.0) + d, np.float32(1.0) - d)
    y = y * np.float32(128.0)
    y = np.round(y)  # round-half-even, same as jnp.round
    return (y / np.float32(128.0)).astype(np.float32)


_cache = {}


def _blockT(a, inner):
    """[rows, D] -> [NCHUNKS, PART, D//PART, NB]: transpose + chunk-block."""
    return np.ascontiguousarray(
        a.reshape(NCHUNKS, NB, inner, PART).transpose(0, 3, 2, 1)
    )


def kernel(**inputs):
    from concourse.bass_utils import run_bass_kernel_spmd

    P = np.asarray(inputs["P"], np.float32)
    Q = np.asarray(inputs["Q"], np.float32)
    S = np.asarray(inputs["S"], np.float32)
    W = np.asarray(inputs["weights"], np.float32)
    bias = np.asarray(inputs["bias"], np.float32)

    wq = _quant_host(W).astype(ml_dtypes.bfloat16)
    # stationary blocks [k*128+p, j*128+m] -> [p, k, j, m], contiguous load
    wq = np.ascontiguousarray(
        wq.reshape(KCH, PART, KOB, PART).transpose(1, 0, 2, 3)
    )
    bq = _quant_host(bias)  # bf16-exact values k/128
    # C = 0.4 - b_q in f32 as a per-partition column per o-block
    ccol = np.ascontiguousarray(
        (np.float32(THR) - bq).reshape(KOB, PART).T
    )
    s8 = S.astype(np.uint8)

    in_maps = []
    clamp = False
    for c in range(NCORES):
        sl = slice(c * BL, (c + 1) * BL)
        x = np.float32(ALPHA) * P[sl] + Q[sl]
        if float(np.max(np.abs(x))) >= 127.5 / 128.0:
            clamp = True
        in_maps.append({
            "p": _blockT(P[sl], KCH),
            "q": _blockT(Q[sl], KCH),
            "s": _blockT(s8[sl], KOB),
            "w": wq,
            "ccol": ccol,
        })

    key = ("nc", clamp)
    if key not in _cache:
        _cache[key] = build_nc(clamp=clamp)
    nc = _cache[key]

    res = run_bass_kernel_spmd(nc, in_maps, list(range(NCORES)))
    _cache["last"] = res  # exec_time_ns etc. when tracing is enabled
    out = np.empty((B, OUT), np.float32)
    for c in range(NCORES):
        o = res.results[c]["o"]  # [NCHUNKS, PART, KOB, NB] u8, o-major
        out[c * BL:(c + 1) * BL] = (
            o.transpose(0, 3, 2, 1).reshape(BL, OUT).astype(np.float32)
        )
    return out


# revision 17
# speedup vs baseline: 1.1169x; 1.1169x over previous
"""Trainium2 Bass kernel for the LIF dense layer (spike output only).

The reference computes
    P_n   = quant8(alpha*P + Q)            (grid 1/128, round-half-even)
    U     = P_n @ quant8(W) + quant8(b) - S
    S_n   = (U > 0.4)
``input_t`` and ``R`` never influence the output (Q_n/U_q are dead,
gamma == 0), so they are never loaded.

All quantized operands are 8-bit integers scaled by 1/128, hence exactly
representable in bf16, and every partial matmul sum is a multiple of 2^-14
below 2^24 -> bf16 matmul with fp32 PSUM accumulation is bit-exact vs the
fp32 reference einsum.  Rounding uses the fp32 magic-number trick
(x + 1.5*2^16) - 1.5*2^16 == round-to-nearest-even onto the 1/128 grid.

Everything runs transposed: the small [512,512] W is the matmul's
stationary operand, P_n^T the moving one, so PSUM holds U^T tiles
[128 o, nb b] and the per-output constant C = 0.4 - b_q becomes a
per-PARTITION scalar.  The whole epilogue is then a single fused DVE op
per tile:  spike^T = (U^T - C_col) is_gt S^T  (subtract + compare +
uint8 narrowing in one pass).  U^T is exact on the 2^-14 grid, C is one
f32 rounding off the real value (< 1e-7), and 0.4f+k*2^-7 sits 2.4e-5
away from the nearest grid point, so decisions match the reference
bit-for-bit (b_q is bf16-exact: k/128, |k|<=127).

The quantizer's clip at +/-127/128 only matters when |alpha*P+Q| >=
127.5/128; the host checks the actual inputs (one cheap fused scan) and
compiles the clip pass in only when needed.

Host-side prep: P, Q, S are shipped transposed and chunk-blocked
([nchunks, 128, ., nb] per core) so the contraction (resp. output) dim
lands on SBUF partitions directly -- no on-chip transposes and 4-16 KiB
contiguous per partition per DMA.  S travels as uint8 and the spike
output returns as blocked uint8 (values are exactly 0/1), quartering
their HBM traffic.

Engine split per chunk: Pool computes alpha*P+Q (fused mult+add), ACT
does the two magic-number adds (the second narrowing to bf16), TensorE
runs 16 N=1024 matmuls, DVE runs the two fused spike ops off PSUM.

Sharding: pure data parallel over the batch dim, 4096 rows per core on 8
NeuronCores; the [512,512] weights / bias are quantized host-side (exact
replication of the reference quantizer) and replicated.
"""

import sys

import numpy as np

sys.path.insert(0, "/opt/trn_rl_repo")

import ml_dtypes

B, IN, OUT = 32768, 512, 512
NCORES = 8
BL = B // NCORES            # rows per core
PART = 128                  # SBUF partitions
KCH = IN // PART            # contraction chunks of 128
KOB = OUT // PART           # output chunks of 128
NB = 512                    # batch columns per pipeline chunk
NCHUNKS = BL // NB
OG = 2                      # o-blocks per PSUM group / fused spike op
# exp(-dt/tau_mem) as computed by XLA fp32 (1 ulp above numpy's expf)
ALPHA = float(np.array(1062312023, np.uint32).view(np.float32))
MAGIC = 98304.0             # 1.5*2^16: fp32 +/- rounds to multiples of 2^-7
QMAX = 127.0 / 128.0
THR = 0.4


def build_nc(bl=BL, nb=NB, clamp=False, enable_asserts=False):
    import concourse.bass as bass
    import concourse.bacc as bacc
    import concourse.mybir as mybir
    from concourse import tile

    OP = mybir.AluOpType
    AF = mybir.ActivationFunctionType
    dt = mybir.dt
    ts = bass.ts

    nchunks = bl // nb

    # Bacc (not plain Bass): its compile() splits multi-sem waits into
    # event semaphores -- TRN2 allows one wait per instruction.
    nc = bacc.Bacc(
        "TRN2",
        target_bir_lowering=False,
        debug=False,
        enable_asserts=enable_asserts,
        num_devices=NCORES,
    )
    p_d = nc.dram_tensor(
        "p", [nchunks, PART, KCH, nb], dt.float32, kind="ExternalInput"
    ).ap()
    q_d = nc.dram_tensor(
        "q", [nchunks, PART, KCH, nb], dt.float32, kind="ExternalInput"
    ).ap()
    s_d = nc.dram_tensor(
        "s", [nchunks, PART, KOB, nb], dt.uint8, kind="ExternalInput"
    ).ap()
    w_d = nc.dram_tensor(
        "w", [PART, KCH, KOB, PART], dt.bfloat16, kind="ExternalInput"
    ).ap()
    c_d = nc.dram_tensor("ccol", [PART, KOB], dt.float32,
                         kind="ExternalInput").ap()
    o_d = nc.dram_tensor(
        "o", [nchunks, PART, KOB, nb], dt.uint8, kind="ExternalOutput"
    ).ap()

    with tile.TileContext(nc) as tc:
        with (
            tc.tile_pool(name="const", bufs=1) as cpool,
            tc.tile_pool(name="io", bufs=4) as iop,
            tc.tile_pool(name="wk", bufs=3) as wkp,
            tc.tile_pool(name="ps", bufs=4, space="PSUM") as psp,
        ):
            w_sb = cpool.tile([PART, KCH, KOB, PART], dt.bfloat16)
            nc.sync.dma_start(out=w_sb[:], in_=w_d[:])
            ccol_sb = cpool.tile([PART, KOB], dt.float32)
            nc.sync.dma_start(out=ccol_sb[:], in_=c_d[:])

            # software-pipelined emission: the elementwise ops for chunk c+1
            # are emitted BEFORE chunk c's spike ops, so the in-order DVE
            # FIFO computes pn(c+1) while the PE is busy with chunk c.
            pn_ts = {}
            s_ts = {}

            def emit_elem(c):
                p_t = iop.tile([PART, KCH, nb], dt.float32, tag="p")
                q_t = iop.tile([PART, KCH, nb], dt.float32, tag="q")
                s_t = iop.tile([PART, KOB, nb], dt.uint8, tag="s")
                nc.sync.dma_start(out=p_t[:], in_=p_d[c])
                nc.scalar.dma_start(out=q_t[:], in_=q_d[c])
                nc.sync.dma_start(out=s_t[:], in_=s_d[c])
                # x = alpha*P + Q  (fp32, per-stage rounding as in the ref)
                nc.vector.scalar_tensor_tensor(
                    out=p_t[:], in0=p_t[:], scalar=ALPHA, in1=q_t[:],
                    op0=OP.mult, op1=OP.add,
                )
                if clamp:
                    # the reference quantizer's clip at +/-127/128
                    nc.gpsimd.tensor_scalar(
                        out=p_t[:], in0=p_t[:], scalar1=QMAX, scalar2=-QMAX,
                        op0=OP.min, op1=OP.max,
                    )
                # round-half-even onto the 1/128 grid (magic-number trick);
                # k/128 is bf16-exact so the output narrows to bf16 exactly
                pn_t = wkp.tile([PART, KCH, nb], dt.bfloat16, tag="pn")
                nc.vector.tensor_scalar(
                    out=pn_t[:], in0=p_t[:], scalar1=MAGIC, scalar2=-MAGIC,
                    op0=OP.add, op1=OP.add,
                )
                pn_ts[c] = pn_t
                s_ts[c] = s_t

            emit_elem(0)
            for c in range(nchunks):
                pn_t = pn_ts.pop(c)
                s_t = s_ts.pop(c)
                sp_t = iop.tile([PART, KOB, nb], dt.uint8, tag="sp")
                spikes = []
                for g in range(KOB // OG):
                    u_ps = psp.tile([PART, OG, nb], dt.float32, tag="u")
                    for jj in range(OG):
                        j = g * OG + jj
                        for k in range(KCH):
                            # one PSUM bank (2 KiB) per matmul -> N<=512
                            for h in range(nb // 512):
                                nc.tensor.matmul(
                                    u_ps[:, jj, ts(h, 512)],
                                    lhsT=w_sb[:, k, j, :],
                                    rhs=pn_t[:, k, ts(h, 512)],
                                    start=(k == 0),
                                    stop=(k == KCH - 1),
                                )
                    spikes.append(u_ps)
                # next chunk's elementwise ops enter the DVE FIFO first
                if c + 1 < nchunks:
                    emit_elem(c + 1)
                # spike^T = (U^T - C_col) > S^T   (one fused DVE op each)
                for g, u_ps in enumerate(spikes):
                    for jj in range(OG):
                        j = g * OG + jj
                        nc.vector.scalar_tensor_tensor(
                            out=sp_t[:, j, :],
                            in0=u_ps[:, jj, :],
                            scalar=ccol_sb[:, j:j + 1],
                            in1=s_t[:, j, :],
                            op0=OP.subtract,
                            op1=OP.is_gt,
                        )
                nc.scalar.dma_start(out=o_d[c], in_=sp_t[:])
    nc.finalize()  # Bacc.compile(): splits multi-sem waits (TRN2 1-wait rule)
    return nc


def _quant_host(x):
    """Exact replica of the reference quant_ste forward pass (fp32)."""
    x = np.asarray(x, np.float32)
    d = np.float32(1.0) / np.float32(128.0)
    y = np.clip(x, np.float32(-1.0) + d, np.float32(1.0) - d)
    y = y * np.float32(128.0)
    y = np.round(y)  # round-half-even, same as jnp.round
    return (y / np.float32(128.0)).astype(np.float32)


_cache = {}


def _blockT(a, inner):
    """[rows, D] -> [NCHUNKS, PART, D//PART, NB]: transpose + chunk-block."""
    return np.ascontiguousarray(
        a.reshape(NCHUNKS, NB, inner, PART).transpose(0, 3, 2, 1)
    )


def kernel(**inputs):
    from concourse.bass_utils import run_bass_kernel_spmd

    P = np.asarray(inputs["P"], np.float32)
    Q = np.asarray(inputs["Q"], np.float32)
    S = np.asarray(inputs["S"], np.float32)
    W = np.asarray(inputs["weights"], np.float32)
    bias = np.asarray(inputs["bias"], np.float32)

    wq = _quant_host(W).astype(ml_dtypes.bfloat16)
    # stationary blocks [k*128+p, j*128+m] -> [p, k, j, m], contiguous load
    wq = np.ascontiguousarray(
        wq.reshape(KCH, PART, KOB, PART).transpose(1, 0, 2, 3)
    )
    bq = _quant_host(bias)  # bf16-exact values k/128
    # C = 0.4 - b_q in f32 as a per-partition column per o-block
    ccol = np.ascontiguousarray(
        (np.float32(THR) - bq).reshape(KOB, PART).T
    )
    s8 = S.astype(np.uint8)

    in_maps = []
    clamp = False
    for c in range(NCORES):
        sl = slice(c * BL, (c + 1) * BL)
        x = np.float32(ALPHA) * P[sl] + Q[sl]
        if float(np.max(np.abs(x))) >= 127.5 / 128.0:
            clamp = True
        in_maps.append({
            "p": _blockT(P[sl], KCH),
            "q": _blockT(Q[sl], KCH),
            "s": _blockT(s8[sl], KOB),
            "w": wq,
            "ccol": ccol,
        })

    key = ("nc", clamp)
    if key not in _cache:
        _cache[key] = build_nc(clamp=clamp)
    nc = _cache[key]

    res = run_bass_kernel_spmd(nc, in_maps, list(range(NCORES)))
    _cache["last"] = res  # exec_time_ns etc. when tracing is enabled
    out = np.empty((B, OUT), np.float32)
    for c in range(NCORES):
        o = res.results[c]["o"]  # [NCHUNKS, PART, KOB, NB] u8, o-major
        out[c * BL:(c + 1) * BL] = (
            o.transpose(0, 3, 2, 1).reshape(BL, OUT).astype(np.float32)
        )
    return out


# revision 18
# speedup vs baseline: 1.1962x; 1.0710x over previous
"""Trainium2 Bass kernel for the LIF dense layer (spike output only).

The reference computes
    P_n   = quant8(alpha*P + Q)            (grid 1/128, round-half-even)
    U     = P_n @ quant8(W) + quant8(b) - S
    S_n   = (U > 0.4)
``input_t`` and ``R`` never influence the output (Q_n/U_q are dead,
gamma == 0), so they are never loaded.

All quantized operands are 8-bit integers scaled by 1/128, hence exactly
representable in bf16, and every partial matmul sum is a multiple of 2^-14
below 2^24 -> bf16 matmul with fp32 PSUM accumulation is bit-exact vs the
fp32 reference einsum.  Rounding uses the fp32 magic-number trick
(x + 1.5*2^16) - 1.5*2^16 == round-to-nearest-even onto the 1/128 grid.

Everything runs transposed: the small [512,512] W is the matmul's
stationary operand, P_n^T the moving one, so PSUM holds U^T tiles
[128 o, nb b] and the per-output constant C = 0.4 - b_q becomes a
per-PARTITION scalar.  The whole epilogue is then a single fused DVE op
per tile:  spike^T = (U^T - C_col) is_gt S^T  (subtract + compare +
uint8 narrowing in one pass).  U^T is exact on the 2^-14 grid, C is one
f32 rounding off the real value (< 1e-7), and 0.4f+k*2^-7 sits 2.4e-5
away from the nearest grid point, so decisions match the reference
bit-for-bit (b_q is bf16-exact: k/128, |k|<=127).

The quantizer's clip at +/-127/128 only matters when |alpha*P+Q| >=
127.5/128; the host checks the actual inputs (one cheap fused scan) and
compiles the clip pass in only when needed.

Host-side prep: P, Q, S are shipped transposed and chunk-blocked
([nchunks, 128, ., nb] per core) so the contraction (resp. output) dim
lands on SBUF partitions directly -- no on-chip transposes and 4-16 KiB
contiguous per partition per DMA.  S travels as uint8 and the spike
output returns as blocked uint8 (values are exactly 0/1), quartering
their HBM traffic.

Engine split per chunk: Pool computes alpha*P+Q (fused mult+add), ACT
does the two magic-number adds (the second narrowing to bf16), TensorE
runs 16 N=1024 matmuls, DVE runs the two fused spike ops off PSUM.

Sharding: pure data parallel over the batch dim, 4096 rows per core on 8
NeuronCores; the [512,512] weights / bias are quantized host-side (exact
replication of the reference quantizer) and replicated.
"""

import sys

import numpy as np

sys.path.insert(0, "/opt/trn_rl_repo")

import ml_dtypes

B, IN, OUT = 32768, 512, 512
NCORES = 8
BL = B // NCORES            # rows per core
PART = 128                  # SBUF partitions
KCH = IN // PART            # contraction chunks of 128
KOB = OUT // PART           # output chunks of 128
NB = 512                    # batch columns per pipeline chunk
NCHUNKS = BL // NB
OG = 2                      # o-blocks per PSUM group / fused spike op
# exp(-dt/tau_mem) as computed by XLA fp32 (1 ulp above numpy's expf)
ALPHA = float(np.array(1062312023, np.uint32).view(np.float32))
MAGIC = 98304.0             # 1.5*2^16: fp32 +/- rounds to multiples of 2^-7
QMAX = 127.0 / 128.0
THR = 0.4


def build_nc(bl=BL, nb=NB, clamp=False, enable_asserts=False):
    import concourse.bass as bass
    import concourse.bacc as bacc
    import concourse.mybir as mybir
    from concourse import tile

    OP = mybir.AluOpType
    AF = mybir.ActivationFunctionType
    dt = mybir.dt
    ts = bass.ts

    nchunks = bl // nb

    # Bacc (not plain Bass): its compile() splits multi-sem waits into
    # event semaphores -- TRN2 allows one wait per instruction.
    nc = bacc.Bacc(
        "TRN2",
        target_bir_lowering=False,
        debug=False,
        enable_asserts=enable_asserts,
        num_devices=NCORES,
    )
    p_d = nc.dram_tensor(
        "p", [nchunks, PART, KCH, nb], dt.float32, kind="ExternalInput"
    ).ap()
    q_d = nc.dram_tensor(
        "q", [nchunks, PART, KCH, nb], dt.float32, kind="ExternalInput"
    ).ap()
    s_d = nc.dram_tensor(
        "s", [nchunks, PART, KOB, nb], dt.uint8, kind="ExternalInput"
    ).ap()
    w_d = nc.dram_tensor(
        "w", [PART, KCH, KOB, PART], dt.bfloat16, kind="ExternalInput"
    ).ap()
    c_d = nc.dram_tensor("ccol", [PART, KOB], dt.float32,
                         kind="ExternalInput").ap()
    o_d = nc.dram_tensor(
        "o", [nchunks, PART, KOB, nb], dt.uint8, kind="ExternalOutput"
    ).ap()

    with tile.TileContext(nc) as tc:
        with (
            tc.tile_pool(name="const", bufs=1) as cpool,
            tc.tile_pool(name="io", bufs=4) as iop,
            tc.tile_pool(name="wk", bufs=3) as wkp,
            tc.tile_pool(name="ps", bufs=4, space="PSUM") as psp,
        ):
            w_sb = cpool.tile([PART, KCH, KOB, PART], dt.bfloat16)
            nc.sync.dma_start(out=w_sb[:], in_=w_d[:])
            ccol_sb = cpool.tile([PART, KOB], dt.float32)
            nc.sync.dma_start(out=ccol_sb[:], in_=c_d[:])

            # software-pipelined emission with 2-chunk lookahead: the
            # elementwise ops for chunk c+2 are emitted BEFORE chunk c's
            # spike ops, so the in-order DVE/ACT FIFOs compute pn(c+2) and
            # T(c+2) while the PE is busy with chunk c.
            pn_ts = {}
            t_ts = {}

            def emit_elem(c):
                p_t = iop.tile([PART, KCH, nb], dt.float32, tag="p")
                q_t = iop.tile([PART, KCH, nb], dt.float32, tag="q")
                s_t = iop.tile([PART, KOB, nb], dt.uint8, tag="s")
                nc.sync.dma_start(out=p_t[:], in_=p_d[c])
                nc.scalar.dma_start(out=q_t[:], in_=q_d[c])
                nc.sync.dma_start(out=s_t[:], in_=s_d[c])
                # x = alpha*P + Q  (fp32, per-stage rounding as in the ref)
                nc.vector.scalar_tensor_tensor(
                    out=p_t[:], in0=p_t[:], scalar=ALPHA, in1=q_t[:],
                    op0=OP.mult, op1=OP.add,
                )
                if clamp:
                    # the reference quantizer's clip at +/-127/128
                    nc.gpsimd.tensor_scalar(
                        out=p_t[:], in0=p_t[:], scalar1=QMAX, scalar2=-QMAX,
                        op0=OP.min, op1=OP.max,
                    )
                # round-half-even onto the 1/128 grid (magic-number trick);
                # k/128 is bf16-exact so the output narrows to bf16 exactly
                pn_t = wkp.tile([PART, KCH, nb], dt.bfloat16, tag="pn")
                nc.vector.tensor_scalar(
                    out=pn_t[:], in0=p_t[:], scalar1=MAGIC, scalar2=-MAGIC,
                    op0=OP.add, op1=OP.add,
                )
                # T = S + (0.4 - b_q): per-partition bias on the ACT engine
                t_t = wkp.tile([PART, KOB, nb], dt.float32, tag="t")
                for j in range(KOB):
                    nc.scalar.activation(
                        t_t[:, j, :], s_t[:, j, :], AF.Identity,
                        bias=ccol_sb[:, j:j + 1],
                    )
                pn_ts[c] = pn_t
                t_ts[c] = t_t

            emit_elem(0)
            emit_elem(1)
            for c in range(nchunks):
                pn_t = pn_ts.pop(c)
                t_t = t_ts.pop(c)
                sp_t = iop.tile([PART, KOB, nb], dt.uint8, tag="sp")
                spikes = []
                for g in range(KOB // OG):
                    u_ps = psp.tile([PART, OG, nb], dt.float32, tag="u")
                    for jj in range(OG):
                        j = g * OG + jj
                        for k in range(KCH):
                            # one PSUM bank (2 KiB) per matmul -> N<=512
                            for h in range(nb // 512):
                                nc.tensor.matmul(
                                    u_ps[:, jj, ts(h, 512)],
                                    lhsT=w_sb[:, k, j, :],
                                    rhs=pn_t[:, k, ts(h, 512)],
                                    start=(k == 0),
                                    stop=(k == KCH - 1),
                                )
                    spikes.append(u_ps)
                # next chunks' elementwise ops enter the DVE FIFO first
                if c + 2 < nchunks:
                    emit_elem(c + 2)
                # spike^T = U^T > T  (plain is_gt per PSUM group)
                for g, u_ps in enumerate(spikes):
                    nc.vector.tensor_tensor(
                        out=sp_t[:, ts(g, OG), :],
                        in0=u_ps[:],
                        in1=t_t[:, ts(g, OG), :],
                        op=OP.is_gt,
                    )
                nc.scalar.dma_start(out=o_d[c], in_=sp_t[:])
    nc.finalize()  # Bacc.compile(): splits multi-sem waits (TRN2 1-wait rule)
    return nc


def _quant_host(x):
    """Exact replica of the reference quant_ste forward pass (fp32)."""
    x = np.asarray(x, np.float32)
    d = np.float32(1.0) / np.float32(128.0)
    y = np.clip(x, np.float32(-1.0) + d, np.float32(1.0) - d)
    y = y * np.float32(128.0)
    y = np.round(y)  # round-half-even, same as jnp.round
    return (y / np.float32(128.0)).astype(np.float32)


_cache = {}


def _blockT(a, inner):
    """[rows, D] -> [NCHUNKS, PART, D//PART, NB]: transpose + chunk-block."""
    return np.ascontiguousarray(
        a.reshape(NCHUNKS, NB, inner, PART).transpose(0, 3, 2, 1)
    )


def kernel(**inputs):
    from concourse.bass_utils import run_bass_kernel_spmd

    P = np.asarray(inputs["P"], np.float32)
    Q = np.asarray(inputs["Q"], np.float32)
    S = np.asarray(inputs["S"], np.float32)
    W = np.asarray(inputs["weights"], np.float32)
    bias = np.asarray(inputs["bias"], np.float32)

    wq = _quant_host(W).astype(ml_dtypes.bfloat16)
    # stationary blocks [k*128+p, j*128+m] -> [p, k, j, m], contiguous load
    wq = np.ascontiguousarray(
        wq.reshape(KCH, PART, KOB, PART).transpose(1, 0, 2, 3)
    )
    bq = _quant_host(bias)  # bf16-exact values k/128
    # C = 0.4 - b_q in f32 as a per-partition column per o-block
    ccol = np.ascontiguousarray(
        (np.float32(THR) - bq).reshape(KOB, PART).T
    )
    s8 = S.astype(np.uint8)

    in_maps = []
    clamp = False
    for c in range(NCORES):
        sl = slice(c * BL, (c + 1) * BL)
        x = np.float32(ALPHA) * P[sl] + Q[sl]
        if float(np.max(np.abs(x))) >= 127.5 / 128.0:
            clamp = True
        in_maps.append({
            "p": _blockT(P[sl], KCH),
            "q": _blockT(Q[sl], KCH),
            "s": _blockT(s8[sl], KOB),
            "w": wq,
            "ccol": ccol,
        })

    key = ("nc", clamp)
    if key not in _cache:
        _cache[key] = build_nc(clamp=clamp)
    nc = _cache[key]

    res = run_bass_kernel_spmd(nc, in_maps, list(range(NCORES)))
    _cache["last"] = res  # exec_time_ns etc. when tracing is enabled
    out = np.empty((B, OUT), np.float32)
    for c in range(NCORES):
        o = res.results[c]["o"]  # [NCHUNKS, PART, KOB, NB] u8, o-major
        out[c * BL:(c + 1) * BL] = (
            o.transpose(0, 3, 2, 1).reshape(BL, OUT).astype(np.float32)
        )
    return out
